# revision 36
# baseline (speedup 1.0000x reference)
"""Trainium2 Bass kernel for nn_Attention (B=4, N=2048, D=768, H=12).

Sharding: batch x head-half across 8 cores (core c -> batch c//2, heads
6*(c%2) .. 6*(c%2)+6). Each core computes its 6 heads' probs
[6,2048,2048], half of its batch's attn_policy rows [1024,2048], and a
partial (6-head) out^T [768,2048]; the host sums the two partials per
batch and adds bo.

Device algorithm per core (all matmuls bf16 -> fp32 PSUM):
  Augmented Q^T/K^T tiles fold the policy multiply and the softmax
  normalizer into the score matmuls, so no on-chip transposes of the
  [N,N] tensors are ever needed:
    even heads (data rows 0-63):  rows 64-65: Q=1, K=ln(pol) hi/lo
                                  rows 66-67: Q=lnr hi/lo, K=1
    odd heads (data rows 64-127): rows 62-63: Q=1, K=ln(pol) hi/lo
                                  rows 60-61: Q=lnr hi/lo, K=1
  where lnr_i = ln(1/(rowsum_i + eps)).
  - pass A (per head, i-tile): scores matmul (K=66) + diagonal patch
    matmuls (cancel ln(pol) on the diagonal), exp on ACT, row-sum +
    probs = a*r + c on DVE, DMA out.
  - pass B (per head pair, j-tile): transposed scores (K=68) -> exp ->
    aT (bf16, normalizer folded in), ctx^T accumulated on PE + rank-1
    eps-correction matmul; finally out^T = Wo~^T.T @ ctx^T.
"""

import os
import sys

for _p in ("/opt/trn_rl_repo", "/root/.axon_site/_ro/trn_rl_repo"):
    if os.path.isdir(_p) and _p not in sys.path:
        sys.path.insert(0, _p)

import numpy as np
import ml_dtypes
from contextlib import ExitStack

import concourse.bass as bass
import concourse.tile as tile
from concourse import mybir
from concourse.bass_utils import run_bass_kernel_spmd
from concourse.masks import make_identity

BFNP = ml_dtypes.bfloat16
F32 = mybir.dt.float32
BF16 = mybir.dt.bfloat16
AF = mybir.ActivationFunctionType
ALU = mybir.AluOpType

B, N, D, H = 4, 2048, 768, 12
DH = 64
HPC = 6            # heads per core
NPAIR = HPC // 2   # head pairs per core
EPS = 1e-6
NT = N // 128      # 16
KT = 7             # projection contraction tiles (768 data + bias row + pad)
LN_EPS_N = float(np.log(EPS / N))


def _emit(ctx: ExitStack, tc: "tile.TileContext"):
    nc = tc.nc

    # ---------------- IO ----------------
    hsT_d = nc.declare_dram_parameter("hsT", [D, N], BF16, isOutput=False)
    wq_d = nc.declare_dram_parameter("wqT", [KT * 128, HPC * DH], BF16, isOutput=False)
    wk_d = nc.declare_dram_parameter("wkT", [KT * 128, HPC * DH], BF16, isOutput=False)
    wv_d = nc.declare_dram_parameter("wvT", [KT * 128, HPC * DH], BF16, isOutput=False)
    wo_d = nc.declare_dram_parameter("woT", [HPC * DH, D], BF16, isOutput=False)
    lnp2_d = nc.declare_dram_parameter("lnp2", [2, N], BF16, isOutput=False)
    dhi_d = nc.declare_dram_parameter("dhi", [128, NT, 128], BF16, isOutput=False)
    dlo_d = nc.declare_dram_parameter("dlo", [128, NT, 128], BF16, isOutput=False)
    pol_sh_d = nc.declare_dram_parameter("pol_sh", [N], F32, isOutput=False)
    adiag_d = nc.declare_dram_parameter("adiag", [128, 8, 128], F32, isOutput=False)

    probs_o = nc.declare_dram_parameter("probs_o", [HPC, N, N], F32, isOutput=True)
    apol_o = nc.declare_dram_parameter("apol_o", [N // 2, N], F32, isOutput=True)
    outT_o = nc.declare_dram_parameter("outT_o", [D, N], F32, isOutput=True)

    # ---------------- constants ----------------
    consts = ctx.enter_context(tc.tile_pool(name="consts", bufs=1))
    keep = ctx.enter_context(tc.tile_pool(name="keep", bufs=1))

    ident_bf = consts.tile([128, 128], BF16)
    make_identity(nc, ident_bf)

    # diag-patch stationary tiles: Dhi[:, t, :] = diag(-lnp_hi[128t:128t+128])
    Dhi = consts.tile([128, NT, 128], BF16)
    nc.sync.dma_start(Dhi, dhi_d[:, :, :])
    Dlo = consts.tile([128, NT, 128], BF16)
    nc.sync.dma_start(Dlo, dlo_d[:, :, :])

    # policy broadcast [128, N] (host pre-rolls so this core's attn_policy
    # diagonal sits at local column 128*k)
    polB = consts.tile([128, N], F32)
    _pap = pol_sh_d[:]
    nc.sync.dma_start(polB, bass.AP(tensor=_pap.tensor, offset=_pap.offset, ap=[[0, 128]] + list(_pap.ap)))

    # patched diagonal blocks for the attn_policy output rows (host-built)
    adiag = consts.tile([128, 8, 128], F32)
    nc.sync.dma_start(adiag, adiag_d[:, :, :])

    onescol = consts.tile([128, 1], BF16)
    nc.vector.memset(onescol, 1.0)
    lneps_col = consts.tile([128, 1], F32)
    nc.vector.memset(lneps_col, LN_EPS_N)

    # ---------------- persistent attention operands ----------------
    # Even heads (h=2m): data rows 0-63; aug rows 64-65: Q=lnr hi/lo, K=1;
    #   rows 66-67: Q=1, K=lnp hi/lo. Contraction slice [0:68].
    # Odd heads (h=2m+1): data rows 64-127; aug rows 0-1: Q=lnr, K=1;
    #   rows 2-3: Q=1, K=lnp; rows 4-63 zero. Contraction slice [0:128].
    # The lnr rows start zeroed, so pass A (which runs before lnr is known)
    # reads them as 0 against K=1 -> no contribution.
    QarrE = keep.tile([128, NPAIR, N], BF16)
    QarrO = keep.tile([128, NPAIR, N], BF16)
    KarrE = keep.tile([128, NPAIR, N], BF16)
    KarrO = keep.tile([128, NPAIR, N], BF16)
    V_sb = keep.tile([128, NT, HPC * DH], BF16)   # [j-part, j-tile, 64h+dh]
    ctxT_sb = keep.tile([128, NPAIR, N], BF16)
    wo_sb = keep.tile([128, HPC * DH // 128, D], BF16)
    rr = keep.tile([128, HPC * NT], F32)          # (rowsum+eps), partition layout
    vsum_sb = keep.tile([1, HPC * DH], BF16)
    c_sb = keep.tile([1, HPC, N], BF16)

    # memset start-partitions must be in {0,32,64,96}; build row patterns
    # by layered overwrites.
    nc.vector.memset(QarrE[64:68, :, :], 1.0)
    nc.vector.memset(QarrE[64:66, :, :], 0.0)     # lnr rows, written per pair
    nc.vector.memset(KarrE[64:66, :, :], 1.0)     # pairs Q lnr rows
    nc.vector.memset(QarrO[0:64, :, :], 0.0)
    nc.vector.memset(QarrO[0:4, :, :], 1.0)
    nc.vector.memset(QarrO[0:2, :, :], 0.0)       # lnr rows, written per pair
    nc.vector.memset(KarrO[0:64, :, :], 0.0)
    nc.vector.memset(KarrO[0:2, :, :], 1.0)       # pairs Q lnr rows
    for m in range(NPAIR):
        nc.sync.dma_start(KarrE[66:68, m, :], lnp2_d[:, :])
        nc.sync.dma_start(KarrO[2:4, m, :], lnp2_d[:, :])

    nc.sync.dma_start(wo_sb, wo_d.rearrange("(t p) d -> p t d", p=128))

    def q_slices(h):
        m, odd = divmod(h, 2)
        if odd:
            return QarrO[0:128, m, :], KarrO[0:128, m, :]
        return QarrE[0:68, m, :], KarrE[0:68, m, :]

    # ---------------- projections ----------------
    with tc.tile_pool(name="proj", bufs=1) as proj, tc.tile_pool(
        name="proj_ps", bufs=2, space="PSUM"
    ) as proj_ps:
        hsT_sb = proj.tile([128, KT, N], BF16)
        nc.sync.dma_start(hsT_sb[:, 0:6, :], hsT_d[:, :].rearrange("(t p) n -> p t n", p=128))
        nc.vector.memset(hsT_sb[:, 6, :], 0.0)
        nc.vector.memset(hsT_sb[0:1, 6, :], 1.0)

        wq_sb = proj.tile([128, KT, HPC * DH], BF16)
        nc.sync.dma_start(wq_sb, wq_d.rearrange("(t p) m -> p t m", p=128))
        wk_sb = proj.tile([128, KT, HPC * DH], BF16)
        nc.sync.dma_start(wk_sb, wk_d.rearrange("(t p) m -> p t m", p=128))
        wv_sb = proj.tile([128, KT, HPC * DH], BF16)
        nc.sync.dma_start(wv_sb, wv_d.rearrange("(t p) m -> p t m", p=128))

        # Q^T and K^T: psum [128, 512] covers head pair m (rows 0-63 even,
        # 64-127 odd); copies are lane-aligned by construction.
        for warr, dstE, dstO in ((wq_sb, QarrE, QarrO), (wk_sb, KarrE, KarrO)):
            for m in range(NPAIR):
                for c in range(N // 512):
                    ps_qk = proj_ps.tile([128, 512], F32, tag="ps_qk")
                    for t in range(KT):
                        nc.tensor.matmul(
                            ps_qk,
                            lhsT=warr[:, t, 128 * m : 128 * m + 128],
                            rhs=hsT_sb[:, t, 512 * c : 512 * c + 512],
                            start=(t == 0),
                            stop=(t == KT - 1),
                        )
                    nc.vector.tensor_copy(dstE[0:64, m, 512 * c : 512 * c + 512], ps_qk[0:64, :])
                    nc.vector.tensor_copy(dstO[64:128, m, 512 * c : 512 * c + 512], ps_qk[64:128, :])

        # V: out[j, dh] tiles
        for jt in range(NT):
            ps_v = proj_ps.tile([128, HPC * DH], F32, tag="ps_v")
            for t in range(KT):
                nc.tensor.matmul(
                    ps_v,
                    lhsT=hsT_sb[:, t, 128 * jt : 128 * jt + 128],
                    rhs=wv_sb[:, t, :],
                    start=(t == 0),
                    stop=(t == KT - 1),
                )
            nc.vector.tensor_copy(V_sb[:, jt, :], ps_v)

        # column sums of V in free layout: [1, 384]
        ps_vs = proj_ps.tile([1, HPC * DH], F32, tag="ps_vs", bufs=1)
        for jt in range(NT):
            nc.tensor.matmul(
                ps_vs,
                lhsT=onescol,
                rhs=V_sb[:, jt, :],
                start=(jt == 0),
                stop=(jt == NT - 1),
            )
        nc.vector.tensor_copy(vsum_sb, ps_vs)

    # ---------------- attention ----------------
    attn_ctx = ctx.enter_context(ExitStack())
    sc_ps = attn_ctx.enter_context(tc.tile_pool(name="sc_ps", bufs=2, space="PSUM"))
    ctx_ps = attn_ctx.enter_context(tc.tile_pool(name="ctx_ps", bufs=1, space="PSUM"))
    ablk_pool = attn_ctx.enter_context(tc.tile_pool(name="ablk_pool", bufs=2))
    probs_pool = attn_ctx.enter_context(tc.tile_pool(name="probs_pool", bufs=4))
    aT_pool = attn_ctx.enter_context(tc.tile_pool(name="aT_pool", bufs=3))
    small = attn_ctx.enter_context(tc.tile_pool(name="small", bufs=6))

    def scores_tile(ps, qa, ka, sl, dtile):
        """2 matmuls producing one [128,1024] slot of scores (+diag patch).
        qa: stationary [K, 128] slice; ka: moving [K, N] source; sl: which
        1024-slot; dtile: diag patch column tile index (or None)."""
        o = 128 * dtile - 1024 * sl if dtile is not None else -1
        for c in range(2):
            has_patch = dtile is not None and 0 <= o - 512 * c < 512
            nc.tensor.matmul(
                ps[:, 512 * c : 512 * c + 512],
                lhsT=qa,
                rhs=ka[:, 1024 * sl + 512 * c : 1024 * sl + 512 * c + 512],
                start=True,
                stop=not has_patch,
            )
        if dtile is not None:
            nc.tensor.matmul(ps[:, o : o + 128], lhsT=Dhi[:, dtile, :], rhs=ident_bf,
                             start=False, stop=False)
            nc.tensor.matmul(ps[:, o : o + 128], lhsT=Dlo[:, dtile, :], rhs=ident_bf,
                             start=False, stop=True)

    def pass_a(h):
        qA, kA = q_slices(h)
        for it in range(NT):
            i0 = 128 * it
            ablk = ablk_pool.tile([128, N], F32, tag="ablk", name="ablk")
            for sl in range(2):
                ps = sc_ps.tile([128, 1024], F32, tag="sc", name="ps_sc")
                scores_tile(ps, qA[:, i0 : i0 + 128], kA, sl,
                            it if (it // 8) == sl else None)
                nc.scalar.activation(ablk[:, 1024 * sl : 1024 * sl + 1024], ps, AF.Exp)
            seps = rr[:, h * NT + it : h * NT + it + 1]
            nc.vector.reduce_sum(seps, ablk, axis=mybir.AxisListType.X)
            nc.vector.tensor_scalar_add(seps, seps, EPS)
            rcol = small.tile([128, 1], F32, tag="rcol", name="rcol")
            nc.vector.reciprocal(rcol, seps)
            ccol = small.tile([128, 1], F32, tag="ccol", name="ccol")
            nc.vector.tensor_scalar_mul(ccol, rcol, EPS / N)
            pt = probs_pool.tile([128, N], F32, tag="probs", name="pt")
            nc.vector.tensor_scalar(pt, ablk, rcol, ccol, ALU.mult, ALU.add)
            nc.sync.dma_start(probs_o[h, i0 : i0 + 128, :], pt)

    def lnr_rows(pr):
        """ln(1/(sum+eps)) hi/lo rows + c row for the heads of pair pr,
        written into the Q aug rows via PE transposes."""
        sl = rr[:, 2 * pr * NT : (2 * pr + 2) * NT]          # [128, 32]
        lnr = small.tile([128, 32], F32, tag="lnr", name="lnr")
        nc.scalar.activation(lnr, sl, AF.Ln)
        nc.vector.tensor_scalar_mul(lnr, lnr, -1.0)
        lnr_hi = small.tile([128, 32], BF16, tag="lnr_hi", name="lnr_hi")
        nc.vector.tensor_copy(lnr_hi, lnr)
        hi32 = small.tile([128, 32], F32, tag="hi32", name="hi32")
        nc.vector.tensor_copy(hi32, lnr_hi)
        lnr_lo = small.tile([128, 32], BF16, tag="lnr_lo", name="lnr_lo")
        nc.vector.tensor_tensor(lnr_lo, lnr, hi32, ALU.subtract)
        crow = small.tile([128, 32], BF16, tag="crow", name="crow")
        nc.scalar.activation(crow, lnr, AF.Exp, bias=lneps_col)
        for src, dstE, dstO in (
            (lnr_hi, QarrE[64:65, pr, :], QarrO[0:1, pr, :]),
            (lnr_lo, QarrE[65:66, pr, :], QarrO[1:2, pr, :]),
            (crow, c_sb[0:1, 2 * pr, :], c_sb[0:1, 2 * pr + 1, :]),
        ):
            pst = sc_ps.tile([32, 128], BF16, tag="sc", name="pst")
            nc.tensor.transpose(pst, src, ident_bf)
            pst_sb = small.tile([32, 128], BF16, tag="pst_sb", name="pst_sb")
            nc.vector.tensor_copy(pst_sb, pst)
            for h2, dst in enumerate((dstE, dstO)):
                nc.sync.dma_start(dst, pst_sb[16 * h2 : 16 * h2 + 16, :])

    def pass_b(pr):
        ctxp = ctx_ps.tile([128, N], F32, tag="ctx", name="ctxp")
        for jt in range(NT):
            j0 = 128 * jt
            for hh in range(2):
                h = 2 * pr + hh
                qB, kB = q_slices(h)
                aT = aT_pool.tile([128, N], BF16, tag="aT", name="aT")
                for sl in range(2):
                    ps = sc_ps.tile([128, 1024], F32, tag="sc", name="ps_sc")
                    scores_tile(ps, kB[:, j0 : j0 + 128], qB, sl,
                                jt if (jt // 8) == sl else None)
                    nc.scalar.activation(aT[:, 1024 * sl : 1024 * sl + 1024], ps, AF.Exp)
                for c4 in range(N // 512):
                    nc.tensor.matmul(
                        ctxp[64 * hh : 64 * hh + 64, 512 * c4 : 512 * c4 + 512],
                        lhsT=V_sb[:, jt, 64 * h : 64 * h + 64],
                        rhs=aT[:, 512 * c4 : 512 * c4 + 512],
                        start=(jt == 0),
                        stop=False,
                        skip_group_check=True,
                    )
        # eps correction: ctx += vsum_dh (x) c_i
        for hh in range(2):
            h = 2 * pr + hh
            for c4 in range(N // 512):
                nc.tensor.matmul(
                    ctxp[64 * hh : 64 * hh + 64, 512 * c4 : 512 * c4 + 512],
                    lhsT=vsum_sb[0:1, 64 * h : 64 * h + 64],
                    rhs=c_sb[0:1, h, 512 * c4 : 512 * c4 + 512],
                    start=False,
                    stop=True,
                    skip_group_check=True,
                )
        nc.vector.tensor_copy(ctxT_sb[:, pr, :], ctxp)

    # spread attn_policy DMAs across the B phases (DMA is idle there)
    apol_sched = [[] for _ in range(NPAIR)]
    for k in range(8):
        apol_sched[k % NPAIR].append(k)

    def write_apol(k):
        c0 = 128 * k
        if c0 > 0:
            nc.sync.dma_start(apol_o[128 * k : 128 * k + 128, 0:c0], polB[:, 0:c0])
        nc.sync.dma_start(apol_o[128 * k : 128 * k + 128, c0 : c0 + 128], adiag[:, k, :])
        if c0 + 128 < N:
            nc.sync.dma_start(apol_o[128 * k : 128 * k + 128, c0 + 128 : N], polB[:, c0 + 128 : N])

    for pr in range(NPAIR):
        pass_a(2 * pr)
        pass_a(2 * pr + 1)
        lnr_rows(pr)
        pass_b(pr)
        for k in apol_sched[pr]:
            write_apol(k)

    attn_ctx.close()

    # ---------------- output projection ----------------
    with tc.tile_pool(name="out_ps", bufs=3, space="PSUM") as out_ps, tc.tile_pool(
        name="out_sb_pool", bufs=3
    ) as out_sb_pool:
        for m in range(D // 128):
            for c4 in range(N // 512):
                pso = out_ps.tile([128, 512], F32, tag="pso", name="pso")
                for t in range(NPAIR):
                    nc.tensor.matmul(
                        pso,
                        lhsT=wo_sb[:, t, 128 * m : 128 * m + 128],
                        rhs=ctxT_sb[:, t, 512 * c4 : 512 * c4 + 512],
                        start=(t == 0),
                        stop=(t == NPAIR - 1),
                    )
                ot_sb = out_sb_pool.tile([128, 512], F32, tag="ot_sb", name="ot_sb")
                nc.vector.tensor_copy(ot_sb, pso)
                nc.sync.dma_start(outT_o[128 * m : 128 * m + 128, 512 * c4 : 512 * c4 + 512], ot_sb)


_PROGRAM = None

# walrus codegen allows only one sync-wait command per instruction
# encoding; excess waits are split onto preceding EventSemaphore
# instructions on the same engine (same semantics: the sequencer executes
# them in order before the instruction).
_WAIT_EXEMPT = {"Call", "UnconditionalBranch"}


def _split_excess_waits(nc):
    for f in nc.m.functions:
        for blk in f.blocks:
            out = []
            changed = False
            for inst in blk.instructions:
                si = inst.sync_info
                limit = None if inst.opcode in _WAIT_EXEMPT else 1
                if si is not None and limit is not None and len(si.on_wait) > limit:
                    waits = list(si.on_wait)
                    excess, kept = waits[:-limit], waits[-limit:]
                    for i, w in enumerate(excess):
                        out.append(
                            mybir.InstEventSemaphore(
                                name=f"{inst.name}-prewait{i}",
                                engine=inst.engine,
                                ins=[],
                                outs=[],
                                sync_info=mybir.SyncInfo(on_wait=[w], on_update=[]),
                            )
                        )
                    inst.sync_info = mybir.SyncInfo(
                        on_wait=kept, on_update=list(si.on_update)
                    )
                    changed = True
                out.append(inst)
            if changed:
                blk.instructions = out


def _get_program(split_waits=True):
    global _PROGRAM
    if _PROGRAM is None:
        nc = bass.Bass()
        with tile.TileContext(nc) as tc:
            with ExitStack() as ctx:
                _emit(ctx, tc)
        if split_waits:
            _split_excess_waits(nc)
        _PROGRAM = nc
    return _PROGRAM


def _prep_core_inputs(hs, pol, Wq, bq, Wk, bk, Wv, bv, Wo, bo, b, half):
    heads = np.arange(HPC * half, HPC * half + HPC)
    hsel = np.concatenate([np.arange(h * DH, (h + 1) * DH) for h in heads])

    def wstack(W, bias, scale):
        w = np.zeros((KT * 128, HPC * DH), np.float32)
        w[:D] = W[hsel].T * scale
        w[D] = bias[hsel] * scale
        return w.astype(BFNP)

    lnp = np.log(np.maximum(pol[b], 1e-30)).astype(np.float32)
    lnp_hi = lnp.astype(BFNP)
    lnp_lo = (lnp - lnp_hi.astype(np.float32)).astype(BFNP)

    idx = np.arange(128)

    def diag_tiles(neg):
        t = np.zeros((NT, 128, 128), BFNP)
        t[:, idx, idx] = neg.reshape(NT, 128)
        return np.ascontiguousarray(t.transpose(1, 0, 2))

    pol_sh = np.roll(pol[b], -(N // 2) * half).astype(np.float32)
    # adiag[p, k, :] = pol_sh patched with 1.0 at the local diagonal col
    adiag = np.empty((128, 8, 128), np.float32)
    for k in range(8):
        blk = np.broadcast_to(pol_sh[128 * k : 128 * k + 128][None, :], (128, 128)).copy()
        blk[idx, idx] = 1.0
        adiag[:, k, :] = blk

    return {
        "hsT": np.ascontiguousarray(hs[b].T).astype(BFNP),
        "wqT": wstack(Wq, bq, 1.0 / 8.0),
        "wkT": wstack(Wk, bk, 1.0),
        "wvT": wstack(Wv, bv, 1.0),
        "woT": np.ascontiguousarray(Wo[:, hsel].T).astype(BFNP),
        "lnp2": np.stack([lnp_hi, lnp_lo]),
        "dhi": diag_tiles(-lnp_hi.astype(np.float32)),
        "dlo": diag_tiles(-lnp_lo.astype(np.float32)),
        "pol_sh": pol_sh,
        "adiag": adiag,
    }


def kernel(hidden_states, policy, Wq, bq, Wk, bk, Wv, bv, Wo, bo):
    hs = np.asarray(hidden_states, np.float32)
    pol = np.asarray(policy, np.float32)
    args = (hs, pol) + tuple(
        np.asarray(x, np.float32) for x in (Wq, bq, Wk, bk, Wv, bv, Wo, bo)
    )

    nc = _get_program()
    in_maps = [_prep_core_inputs(*args, b=c // 2, half=c % 2) for c in range(8)]
    res = run_bass_kernel_spmd(nc, in_maps, list(range(8)))

    probs = np.empty((B, H, N, N), np.float32)
    apol = np.empty((B, 1, N, N), np.float32)
    out = np.empty((B, N, D), np.float32)
    bo32 = np.asarray(bo, np.float32)
    for b in range(B):
        r0, r1 = res.results[2 * b], res.results[2 * b + 1]
        probs[b, 0:HPC] = r0["probs_o"]
        probs[b, HPC:H] = r1["probs_o"]
        apol[b, 0, 0 : N // 2, :] = r0["apol_o"]
        # the half-1 core wrote its rows with columns rolled left by N/2
        apol[b, 0, N // 2 :, N // 2 :] = r1["apol_o"][:, 0 : N // 2]
        apol[b, 0, N // 2 :, 0 : N // 2] = r1["apol_o"][:, N // 2 :]
        out[b] = (r0["outT_o"].astype(np.float32) + r1["outT_o"]).T + bo32[None, :]
    return out, probs, apol


# revision 38
# speedup vs baseline: 1.2048x; 1.2048x over previous
"""Trainium2 Bass kernel for nn_Attention (B=4, N=2048, D=768, H=12).

Sharding: batch x head-half across 8 cores (core c -> batch c//2, heads
6*(c%2) .. 6*(c%2)+6). Each core computes its 6 heads' probs
[6,2048,2048], half of its batch's attn_policy rows [1024,2048], and a
partial (6-head) out^T [768,2048]; the host sums the two partials per
batch and adds bo.

Device algorithm per core (all matmuls bf16 -> fp32 PSUM):
  Augmented Q^T/K^T tiles fold the policy multiply and the softmax
  normalizer into the score matmuls, so no on-chip transposes of the
  [N,N] tensors are ever needed:
    even heads (data rows 0-63):  rows 64-65: Q=1, K=ln(pol) hi/lo
                                  rows 66-67: Q=lnr hi/lo, K=1
    odd heads (data rows 64-127): rows 62-63: Q=1, K=ln(pol) hi/lo
                                  rows 60-61: Q=lnr hi/lo, K=1
  where lnr_i = ln(1/(rowsum_i + eps)).
  - pass A (per head, i-tile): scores matmul (K=66) + diagonal patch
    matmuls (cancel ln(pol) on the diagonal), exp on ACT, row-sum +
    probs = a*r + c on DVE, DMA out.
  - pass B (per head pair, j-tile): transposed scores (K=68) -> exp ->
    aT (bf16, normalizer folded in), ctx^T accumulated on PE + rank-1
    eps-correction matmul; finally out^T = Wo~^T.T @ ctx^T.
"""

import os
import sys

for _p in ("/opt/trn_rl_repo", "/root/.axon_site/_ro/trn_rl_repo"):
    if os.path.isdir(_p) and _p not in sys.path:
        sys.path.insert(0, _p)

import numpy as np
import ml_dtypes
from contextlib import ExitStack

import concourse.bass as bass
import concourse.tile as tile
from concourse import mybir
from concourse.bass_utils import run_bass_kernel_spmd
from concourse.masks import make_identity

BFNP = ml_dtypes.bfloat16
F32 = mybir.dt.float32
BF16 = mybir.dt.bfloat16
AF = mybir.ActivationFunctionType
ALU = mybir.AluOpType

B, N, D, H = 4, 2048, 768, 12
DH = 64
HPC = 6            # heads per core
NPAIR = HPC // 2   # head pairs per core
EPS = 1e-6
NT = N // 128      # 16
KT = 7             # projection contraction tiles (768 data + bias row + pad)
LN_EPS_N = float(np.log(EPS / N))


def _emit(ctx: ExitStack, tc: "tile.TileContext"):
    nc = tc.nc

    # ---------------- IO ----------------
    hsT_d = nc.declare_dram_parameter("hsT", [D, N], BF16, isOutput=False)
    wq_d = nc.declare_dram_parameter("wqT", [KT * 128, HPC * DH], BF16, isOutput=False)
    wk_d = nc.declare_dram_parameter("wkT", [KT * 128, HPC * DH], BF16, isOutput=False)
    wv_d = nc.declare_dram_parameter("wvT", [KT * 128, HPC * DH], BF16, isOutput=False)
    wo_d = nc.declare_dram_parameter("woT", [HPC * DH, D], BF16, isOutput=False)
    lnp2_d = nc.declare_dram_parameter("lnp2", [2, N], BF16, isOutput=False)
    dhi_d = nc.declare_dram_parameter("dhi", [128, NT, 128], BF16, isOutput=False)
    dlo_d = nc.declare_dram_parameter("dlo", [128, NT, 128], BF16, isOutput=False)
    pol_sh_d = nc.declare_dram_parameter("pol_sh", [N], F32, isOutput=False)
    adiag_d = nc.declare_dram_parameter("adiag", [128, 8, 128], F32, isOutput=False)

    probs_o = nc.declare_dram_parameter("probs_o", [HPC, N, N], F32, isOutput=True)
    apol_o = nc.declare_dram_parameter("apol_o", [N // 2, N], F32, isOutput=True)
    outT_o = nc.declare_dram_parameter("outT_o", [D, N], F32, isOutput=True)

    # ---------------- constants ----------------
    consts = ctx.enter_context(tc.tile_pool(name="consts", bufs=1))
    keep = ctx.enter_context(tc.tile_pool(name="keep", bufs=1))

    ident_bf = consts.tile([128, 128], BF16)
    make_identity(nc, ident_bf)

    # diag-patch stationary tiles: Dhi[:, t, :] = diag(-lnp_hi[128t:128t+128])
    Dhi = consts.tile([128, NT, 128], BF16)
    nc.sync.dma_start(Dhi, dhi_d[:, :, :])
    Dlo = consts.tile([128, NT, 128], BF16)
    nc.sync.dma_start(Dlo, dlo_d[:, :, :])

    # policy broadcast [128, N] (host pre-rolls so this core's attn_policy
    # diagonal sits at local column 128*k)
    polB = consts.tile([128, N], F32)
    _pap = pol_sh_d[:]
    nc.sync.dma_start(polB, bass.AP(tensor=_pap.tensor, offset=_pap.offset, ap=[[0, 128]] + list(_pap.ap)))

    # patched diagonal blocks for the attn_policy output rows (host-built)
    adiag = consts.tile([128, 8, 128], F32)
    nc.sync.dma_start(adiag, adiag_d[:, :, :])

    onescol = consts.tile([128, 1], BF16)
    nc.vector.memset(onescol, 1.0)
    lneps_col = consts.tile([128, 1], F32)
    nc.vector.memset(lneps_col, LN_EPS_N)

    # ---------------- persistent attention operands ----------------
    # Even heads (h=2m): data rows 0-63; aug rows 64-65: Q=lnr hi/lo, K=1;
    #   rows 66-67: Q=1, K=lnp hi/lo. Contraction slice [0:68].
    # Odd heads (h=2m+1): data rows 64-127; aug rows 0-1: Q=lnr, K=1;
    #   rows 2-3: Q=1, K=lnp; rows 4-63 zero. Contraction slice [0:128].
    # The lnr rows start zeroed, so pass A (which runs before lnr is known)
    # reads them as 0 against K=1 -> no contribution.
    QarrE = keep.tile([128, NPAIR, N], BF16)
    QarrO = keep.tile([128, NPAIR, N], BF16)
    KarrE = keep.tile([128, NPAIR, N], BF16)
    KarrO = keep.tile([128, NPAIR, N], BF16)
    V_sb = keep.tile([128, NT, HPC * DH], BF16)   # [j-part, j-tile, 64h+dh]
    ctxT_sb = keep.tile([128, NPAIR, N], BF16)
    wo_sb = keep.tile([128, HPC * DH // 128, D], BF16)
    rr = keep.tile([128, HPC * NT], F32)          # (rowsum+eps), partition layout
    vsum_sb = keep.tile([1, HPC * DH], BF16)
    c_sb = keep.tile([1, HPC, N], BF16)

    # memset start-partitions must be in {0,32,64,96}; build row patterns
    # by layered overwrites.
    nc.vector.memset(QarrE[64:68, :, :], 1.0)
    nc.vector.memset(QarrE[64:66, :, :], 0.0)     # lnr rows, written per pair
    nc.vector.memset(KarrE[64:66, :, :], 1.0)     # pairs Q lnr rows
    nc.vector.memset(QarrO[0:64, :, :], 0.0)
    nc.vector.memset(QarrO[0:4, :, :], 1.0)
    nc.vector.memset(QarrO[0:2, :, :], 0.0)       # lnr rows, written per pair
    nc.vector.memset(KarrO[0:64, :, :], 0.0)
    nc.vector.memset(KarrO[0:2, :, :], 1.0)       # pairs Q lnr rows
    for m in range(NPAIR):
        nc.sync.dma_start(KarrE[66:68, m, :], lnp2_d[:, :])
        nc.sync.dma_start(KarrO[2:4, m, :], lnp2_d[:, :])

    nc.sync.dma_start(wo_sb, wo_d.rearrange("(t p) d -> p t d", p=128))

    def q_slices(h):
        m, odd = divmod(h, 2)
        if odd:
            return QarrO[0:128, m, :], KarrO[0:128, m, :]
        return QarrE[0:68, m, :], KarrE[0:68, m, :]

    # ---------------- projections ----------------
    with tc.tile_pool(name="proj", bufs=1) as proj, tc.tile_pool(
        name="proj_ps", bufs=2, space="PSUM"
    ) as proj_ps:
        hsT_sb = proj.tile([128, KT, N], BF16)
        nc.sync.dma_start(hsT_sb[:, 0:6, :], hsT_d[:, :].rearrange("(t p) n -> p t n", p=128))
        nc.vector.memset(hsT_sb[:, 6, :], 0.0)
        nc.vector.memset(hsT_sb[0:1, 6, :], 1.0)

        wq_sb = proj.tile([128, KT, HPC * DH], BF16)
        nc.sync.dma_start(wq_sb, wq_d.rearrange("(t p) m -> p t m", p=128))
        wk_sb = proj.tile([128, KT, HPC * DH], BF16)
        nc.sync.dma_start(wk_sb, wk_d.rearrange("(t p) m -> p t m", p=128))
        wv_sb = proj.tile([128, KT, HPC * DH], BF16)
        nc.sync.dma_start(wv_sb, wv_d.rearrange("(t p) m -> p t m", p=128))

        # Q^T and K^T: psum [128, 512] covers head pair m (rows 0-63 even,
        # 64-127 odd); copies are lane-aligned by construction.
        for warr, dstE, dstO in ((wq_sb, QarrE, QarrO), (wk_sb, KarrE, KarrO)):
            for m in range(NPAIR):
                for c in range(N // 512):
                    ps_qk = proj_ps.tile([128, 512], F32, tag="ps_qk")
                    for t in range(KT):
                        nc.tensor.matmul(
                            ps_qk,
                            lhsT=warr[:, t, 128 * m : 128 * m + 128],
                            rhs=hsT_sb[:, t, 512 * c : 512 * c + 512],
                            start=(t == 0),
                            stop=(t == KT - 1),
                        )
                    nc.vector.tensor_copy(dstE[0:64, m, 512 * c : 512 * c + 512], ps_qk[0:64, :])
                    nc.vector.tensor_copy(dstO[64:128, m, 512 * c : 512 * c + 512], ps_qk[64:128, :])

        # V: out[j, dh] tiles
        for jt in range(NT):
            ps_v = proj_ps.tile([128, HPC * DH], F32, tag="ps_v")
            for t in range(KT):
                nc.tensor.matmul(
                    ps_v,
                    lhsT=hsT_sb[:, t, 128 * jt : 128 * jt + 128],
                    rhs=wv_sb[:, t, :],
                    start=(t == 0),
                    stop=(t == KT - 1),
                )
            nc.vector.tensor_copy(V_sb[:, jt, :], ps_v)

        # column sums of V in free layout: [1, 384]
        ps_vs = proj_ps.tile([1, HPC * DH], F32, tag="ps_vs", bufs=1)
        for jt in range(NT):
            nc.tensor.matmul(
                ps_vs,
                lhsT=onescol,
                rhs=V_sb[:, jt, :],
                start=(jt == 0),
                stop=(jt == NT - 1),
            )
        nc.vector.tensor_copy(vsum_sb, ps_vs)

    # ---------------- attention ----------------
    attn_ctx = ctx.enter_context(ExitStack())
    sc_ps = attn_ctx.enter_context(tc.tile_pool(name="sc_ps", bufs=2, space="PSUM"))
    ctx_ps = attn_ctx.enter_context(tc.tile_pool(name="ctx_ps", bufs=1, space="PSUM"))
    ablk_pool = attn_ctx.enter_context(tc.tile_pool(name="ablk_pool", bufs=2))
    probs_pool = attn_ctx.enter_context(tc.tile_pool(name="probs_pool", bufs=4))
    aT_pool = attn_ctx.enter_context(tc.tile_pool(name="aT_pool", bufs=3))
    small = attn_ctx.enter_context(tc.tile_pool(name="small", bufs=6))

    def scores_tile(ps, qa, ka, sl, dtile):
        """2 matmuls producing one [128,1024] slot of scores (+diag patch).
        qa: stationary [K, 128] slice; ka: moving [K, N] source; sl: which
        1024-slot; dtile: diag patch column tile index (or None)."""
        o = 128 * dtile - 1024 * sl if dtile is not None else -1
        for c in range(2):
            has_patch = dtile is not None and 0 <= o - 512 * c < 512
            nc.tensor.matmul(
                ps[:, 512 * c : 512 * c + 512],
                lhsT=qa,
                rhs=ka[:, 1024 * sl + 512 * c : 1024 * sl + 512 * c + 512],
                start=True,
                stop=not has_patch,
            )
        if dtile is not None:
            nc.tensor.matmul(ps[:, o : o + 128], lhsT=Dhi[:, dtile, :], rhs=ident_bf,
                             start=False, stop=False)
            nc.tensor.matmul(ps[:, o : o + 128], lhsT=Dlo[:, dtile, :], rhs=ident_bf,
                             start=False, stop=True)

    def a_unit(h, it):
        qA, kA = q_slices(h)
        i0 = 128 * it
        ablk = ablk_pool.tile([128, N], F32, tag="ablk", name="ablk")
        acc = small.tile([128, 2], F32, tag="acc", name="acc")
        for sl in range(2):
            ps = sc_ps.tile([128, 1024], F32, tag="sc", name="ps_sc")
            scores_tile(ps, qA[:, i0 : i0 + 128], kA, sl,
                        it if (it // 8) == sl else None)
            nc.scalar.activation(
                ablk[:, 1024 * sl : 1024 * sl + 1024], ps, AF.Exp,
                accum_out=acc[:, sl : sl + 1],
            )
        seps = rr[:, h * NT + it : h * NT + it + 1]
        nc.vector.tensor_tensor(seps, acc[:, 0:1], acc[:, 1:2], ALU.add)
        nc.vector.tensor_scalar_add(seps, seps, EPS)
        rcol = small.tile([128, 1], F32, tag="rcol", name="rcol")
        nc.vector.reciprocal(rcol, seps)
        ccol = small.tile([128, 1], F32, tag="ccol", name="ccol")
        nc.vector.tensor_scalar_mul(ccol, rcol, EPS / N)
        pt = probs_pool.tile([128, N], F32, tag="probs", name="pt")
        nc.vector.tensor_scalar(pt, ablk, rcol, ccol, ALU.mult, ALU.add)
        nc.sync.dma_start(probs_o[h, i0 : i0 + 128, :], pt)

    def lnr_rows(pr):
        """ln(1/(sum+eps)) hi/lo rows + c row for the heads of pair pr,
        written into the Q aug rows via PE transposes."""
        sl = rr[:, 2 * pr * NT : (2 * pr + 2) * NT]          # [128, 32]
        lnr = small.tile([128, 32], F32, tag="lnr", name="lnr")
        nc.scalar.activation(lnr, sl, AF.Ln)
        nc.vector.tensor_scalar_mul(lnr, lnr, -1.0)
        lnr_hi = small.tile([128, 32], BF16, tag="lnr_hi", name="lnr_hi")
        nc.vector.tensor_copy(lnr_hi, lnr)
        hi32 = small.tile([128, 32], F32, tag="hi32", name="hi32")
        nc.vector.tensor_copy(hi32, lnr_hi)
        lnr_lo = small.tile([128, 32], BF16, tag="lnr_lo", name="lnr_lo")
        nc.vector.tensor_tensor(lnr_lo, lnr, hi32, ALU.subtract)
        crow = small.tile([128, 32], BF16, tag="crow", name="crow")
        nc.scalar.activation(crow, lnr, AF.Exp, bias=lneps_col)
        for src, dstE, dstO in (
            (lnr_hi, QarrE[64:65, pr, :], QarrO[0:1, pr, :]),
            (lnr_lo, QarrE[65:66, pr, :], QarrO[1:2, pr, :]),
            (crow, c_sb[0:1, 2 * pr, :], c_sb[0:1, 2 * pr + 1, :]),
        ):
            pst = sc_ps.tile([32, 128], BF16, tag="sc", name="pst")
            nc.tensor.transpose(pst, src, ident_bf)
            pst_sb = small.tile([32, 128], BF16, tag="pst_sb", name="pst_sb")
            nc.vector.tensor_copy(pst_sb, pst)
            for h2, dst in enumerate((dstE, dstO)):
                nc.sync.dma_start(dst, pst_sb[16 * h2 : 16 * h2 + 16, :])

    def b_unit(pr, jt, ctxp):
        j0 = 128 * jt
        for hh in range(2):
            h = 2 * pr + hh
            qB, kB = q_slices(h)
            aT = aT_pool.tile([128, N], BF16, tag="aT", name="aT")
            for sl in range(2):
                ps = sc_ps.tile([128, 1024], F32, tag="sc", name="ps_sc")
                scores_tile(ps, kB[:, j0 : j0 + 128], qB, sl,
                            jt if (jt // 8) == sl else None)
                nc.scalar.activation(aT[:, 1024 * sl : 1024 * sl + 1024], ps, AF.Exp)
            for c4 in range(N // 512):
                nc.tensor.matmul(
                    ctxp[64 * hh : 64 * hh + 64, 512 * c4 : 512 * c4 + 512],
                    lhsT=V_sb[:, jt, 64 * h : 64 * h + 64],
                    rhs=aT[:, 512 * c4 : 512 * c4 + 512],
                    start=(jt == 0),
                    stop=False,
                    skip_group_check=True,
                )

    def b_tail(pr, ctxp):
        # eps correction: ctx += vsum_dh (x) c_i
        for hh in range(2):
            h = 2 * pr + hh
            for c4 in range(N // 512):
                nc.tensor.matmul(
                    ctxp[64 * hh : 64 * hh + 64, 512 * c4 : 512 * c4 + 512],
                    lhsT=vsum_sb[0:1, 64 * h : 64 * h + 64],
                    rhs=c_sb[0:1, h, 512 * c4 : 512 * c4 + 512],
                    start=False,
                    stop=True,
                    skip_group_check=True,
                )
        nc.vector.tensor_copy(ctxT_sb[:, pr, :], ctxp)

    def write_apol(k):
        c0 = 128 * k
        if c0 > 0:
            nc.sync.dma_start(apol_o[128 * k : 128 * k + 128, 0:c0], polB[:, 0:c0])
        nc.sync.dma_start(apol_o[128 * k : 128 * k + 128, c0 : c0 + 128], adiag[:, k, :])
        if c0 + 128 < N:
            nc.sync.dma_start(apol_o[128 * k : 128 * k + 128, c0 + 128 : N], polB[:, c0 + 128 : N])

    # Interleave pair p's pass A with pair p-1's pass B at tile granularity
    # so ACT/PE/DVE/DMA stay busy simultaneously; pass B of pair p only
    # depends on lnr(p), emitted after pair p's A units.
    apol_left = list(range(8))
    ctx_tiles = {}
    for p in range(NPAIR + 1):
        a_units = []
        if p < NPAIR:
            a_units = [(2 * p, it) for it in range(NT)] + [(2 * p + 1, it) for it in range(NT)]
        b_units = []
        if p > 0:
            ctx_tiles[p - 1] = ctx_ps.tile([128, N], F32, tag="ctx", name="ctxp")
            b_units = [(p - 1, jt) for jt in range(NT)]
        na, nb = len(a_units), len(b_units)
        ia = ib = 0
        while ia < na or ib < nb:
            # pace A twice as fast as B (32 A units vs 16 B units per round)
            if ia < na:
                a_unit(*a_units[ia]); ia += 1
            if ia < na:
                a_unit(*a_units[ia]); ia += 1
            if ib < nb:
                b_unit(b_units[ib][0], b_units[ib][1], ctx_tiles[p - 1]); ib += 1
                if apol_left and ib % 6 == 0:
                    write_apol(apol_left.pop(0))
        if p > 0:
            b_tail(p - 1, ctx_tiles.pop(p - 1))
        if p < NPAIR:
            lnr_rows(p)
    while apol_left:
        write_apol(apol_left.pop(0))

    attn_ctx.close()

    # ---------------- output projection ----------------
    with tc.tile_pool(name="out_ps", bufs=3, space="PSUM") as out_ps, tc.tile_pool(
        name="out_sb_pool", bufs=3
    ) as out_sb_pool:
        for m in range(D // 128):
            for c4 in range(N // 512):
                pso = out_ps.tile([128, 512], F32, tag="pso", name="pso")
                for t in range(NPAIR):
                    nc.tensor.matmul(
                        pso,
                        lhsT=wo_sb[:, t, 128 * m : 128 * m + 128],
                        rhs=ctxT_sb[:, t, 512 * c4 : 512 * c4 + 512],
                        start=(t == 0),
                        stop=(t == NPAIR - 1),
                    )
                ot_sb = out_sb_pool.tile([128, 512], F32, tag="ot_sb", name="ot_sb")
                nc.vector.tensor_copy(ot_sb, pso)
                nc.sync.dma_start(outT_o[128 * m : 128 * m + 128, 512 * c4 : 512 * c4 + 512], ot_sb)


_PROGRAM = None

# walrus codegen allows only one sync-wait command per instruction
# encoding; excess waits are split onto preceding EventSemaphore
# instructions on the same engine (same semantics: the sequencer executes
# them in order before the instruction).
_WAIT_EXEMPT = {"Call", "UnconditionalBranch"}


def _split_excess_waits(nc):
    for f in nc.m.functions:
        for blk in f.blocks:
            out = []
            changed = False
            for inst in blk.instructions:
                si = inst.sync_info
                limit = None if inst.opcode in _WAIT_EXEMPT else 1
                if si is not None and limit is not None and len(si.on_wait) > limit:
                    waits = list(si.on_wait)
                    excess, kept = waits[:-limit], waits[-limit:]
                    for i, w in enumerate(excess):
                        out.append(
                            mybir.InstEventSemaphore(
                                name=f"{inst.name}-prewait{i}",
                                engine=inst.engine,
                                ins=[],
                                outs=[],
                                sync_info=mybir.SyncInfo(on_wait=[w], on_update=[]),
                            )
                        )
                    inst.sync_info = mybir.SyncInfo(
                        on_wait=kept, on_update=list(si.on_update)
                    )
                    changed = True
                out.append(inst)
            if changed:
                blk.instructions = out


def _get_program(split_waits=True):
    global _PROGRAM
    if _PROGRAM is None:
        nc = bass.Bass()
        with tile.TileContext(nc) as tc:
            with ExitStack() as ctx:
                _emit(ctx, tc)
        if split_waits:
            _split_excess_waits(nc)
        _PROGRAM = nc
    return _PROGRAM


def _prep_core_inputs(hs, pol, Wq, bq, Wk, bk, Wv, bv, Wo, bo, b, half):
    heads = np.arange(HPC * half, HPC * half + HPC)
    hsel = np.concatenate([np.arange(h * DH, (h + 1) * DH) for h in heads])

    def wstack(W, bias, scale):
        w = np.zeros((KT * 128, HPC * DH), np.float32)
        w[:D] = W[hsel].T * scale
        w[D] = bias[hsel] * scale
        return w.astype(BFNP)

    lnp = np.log(np.maximum(pol[b], 1e-30)).astype(np.float32)
    lnp_hi = lnp.astype(BFNP)
    lnp_lo = (lnp - lnp_hi.astype(np.float32)).astype(BFNP)

    idx = np.arange(128)

    def diag_tiles(neg):
        t = np.zeros((NT, 128, 128), BFNP)
        t[:, idx, idx] = neg.reshape(NT, 128)
        return np.ascontiguousarray(t.transpose(1, 0, 2))

    pol_sh = np.roll(pol[b], -(N // 2) * half).astype(np.float32)
    # adiag[p, k, :] = pol_sh patched with 1.0 at the local diagonal col
    adiag = np.empty((128, 8, 128), np.float32)
    for k in range(8):
        blk = np.broadcast_to(pol_sh[128 * k : 128 * k + 128][None, :], (128, 128)).copy()
        blk[idx, idx] = 1.0
        adiag[:, k, :] = blk

    return {
        "hsT": np.ascontiguousarray(hs[b].T).astype(BFNP),
        "wqT": wstack(Wq, bq, 1.0 / 8.0),
        "wkT": wstack(Wk, bk, 1.0),
        "wvT": wstack(Wv, bv, 1.0),
        "woT": np.ascontiguousarray(Wo[:, hsel].T).astype(BFNP),
        "lnp2": np.stack([lnp_hi, lnp_lo]),
        "dhi": diag_tiles(-lnp_hi.astype(np.float32)),
        "dlo": diag_tiles(-lnp_lo.astype(np.float32)),
        "pol_sh": pol_sh,
        "adiag": adiag,
    }


def kernel(hidden_states, policy, Wq, bq, Wk, bk, Wv, bv, Wo, bo):
    hs = np.asarray(hidden_states, np.float32)
    pol = np.asarray(policy, np.float32)
    args = (hs, pol) + tuple(
        np.asarray(x, np.float32) for x in (Wq, bq, Wk, bk, Wv, bv, Wo, bo)
    )

    nc = _get_program()
    in_maps = [_prep_core_inputs(*args, b=c // 2, half=c % 2) for c in range(8)]
    res = run_bass_kernel_spmd(nc, in_maps, list(range(8)))

    probs = np.empty((B, H, N, N), np.float32)
    apol = np.empty((B, 1, N, N), np.float32)
    out = np.empty((B, N, D), np.float32)
    bo32 = np.asarray(bo, np.float32)
    for b in range(B):
        r0, r1 = res.results[2 * b], res.results[2 * b + 1]
        probs[b, 0:HPC] = r0["probs_o"]
        probs[b, HPC:H] = r1["probs_o"]
        apol[b, 0, 0 : N // 2, :] = r0["apol_o"]
        # the half-1 core wrote its rows with columns rolled left by N/2
        apol[b, 0, N // 2 :, N // 2 :] = r1["apol_o"][:, 0 : N // 2]
        apol[b, 0, N // 2 :, 0 : N // 2] = r1["apol_o"][:, N // 2 :]
        out[b] = (r0["outT_o"].astype(np.float32) + r1["outT_o"]).T + bo32[None, :]
    return out, probs, apol
